# revision 21
# baseline (speedup 1.0000x reference)
import sys

sys.path.insert(0, "/opt/trn_rl_repo")

import numpy as np

N_GAUSS = 1024
IMG = 128
NB = 8          # gaussian blocks of 128
NP_ = 512       # pixels per matmul tile (one PSUM bank)
P_CORE = 2048   # pixels per core (16 rows x 128 cols)
N_CORES = 8

# static (call-invariant) tensor layout: free-dim offsets in (128, STATIC_F)
O_BAS = 0                       # basis rows 0..5, P_CORE wide
O_TRI = O_BAS + P_CORE          # strict-upper mask, 128 partitions
O_ON1 = O_TRI + 128             # ones row (partition 0, 128 wide)
O_ONC = O_ON1 + 128             # ones column (128 partitions, 1 wide)
STATIC_F = O_ONC + 1

_state = {}


def _quat_to_rot(q):
    q = q / np.linalg.norm(q, axis=1, keepdims=True)
    w, x, y, z = q[:, 0], q[:, 1], q[:, 2], q[:, 3]
    R = np.stack([
        np.stack([1 - 2 * (y * y + z * z), 2 * (x * y - w * z), 2 * (x * z + w * y)], -1),
        np.stack([2 * (x * y + w * z), 1 - 2 * (x * x + z * z), 2 * (y * z - w * x)], -1),
        np.stack([2 * (x * z - w * y), 2 * (y * z + w * x), 1 - 2 * (x * x + y * y)], -1),
    ], -2)
    return R.astype(np.float32)


def _host_prep(camera_poses, positions, scales, rotations, opacity, features):
    pose = np.asarray(camera_poses, np.float32)[0]
    positions = np.asarray(positions, np.float32)
    scales = np.asarray(scales, np.float32)
    rotations = np.asarray(rotations, np.float32)
    opacity = np.asarray(opacity, np.float32)
    features = np.asarray(features, np.float32)
    N = positions.shape[0]

    hom = np.concatenate([positions, np.ones((N, 1), np.float32)], 1)      # (N,4)
    cam = hom @ pose.T                                                     # (N,4)
    depths = cam[:, 2]
    px = cam[:, 0] / depths
    py = cam[:, 1] / depths

    R = _quat_to_rot(rotations)                                            # (N,3,3)
    s2 = (scales * scales)[:, None, :]                                     # (N,1,3)
    cov3d = np.einsum('nij,nkj->nik', R * s2, R)                           # (N,3,3)

    x, y, z = cam[:, 0], cam[:, 1], depths
    zinv = 1.0 / z
    Jp = np.zeros((N, 2, 3), np.float32)
    Jp[:, 0, 0] = zinv
    Jp[:, 0, 2] = -x * zinv * zinv
    Jp[:, 1, 1] = zinv
    Jp[:, 1, 2] = -y * zinv * zinv
    Wc = pose[:3, :3]
    J = Jp @ Wc                                                            # (N,2,3)
    cov2d = np.einsum('nij,njk,nlk->nil', J, cov3d, J)                     # (N,2,2)

    a, b = cov2d[:, 0, 0], cov2d[:, 0, 1]
    c, d = cov2d[:, 1, 0], cov2d[:, 1, 1]
    det = a * d - b * c
    i00, i01, i10, i11 = d / det, -b / det, -c / det, a / det

    order = np.argsort(-depths, kind='stable')
    i00, i11 = i00[order], i11[order]
    s = (i01 + i10)[order]
    px, py = px[order], py[order]
    alp = np.maximum(opacity[order, 0], 1e-37)
    col = features[order]                                                  # (N,3)

    # logits = -0.5*m + ln(alpha) as quadratic in (gx, gy):
    #   A gx^2 + B gx gy + C gy^2 + D gx + E gy + F
    coeff = np.empty((6, N), np.float32)
    coeff[0] = -0.5 * i00
    coeff[1] = -0.5 * s
    coeff[2] = -0.5 * i11
    coeff[3] = i00 * px + 0.5 * s * py
    coeff[4] = 0.5 * s * px + i11 * py
    coeff[5] = -0.5 * (i00 * px * px + s * px * py + i11 * py * py) + np.log(alp)

    colT = np.zeros((128, 3 * NB), np.float32)
    for k in range(NB):
        colT[:, 3 * k:3 * k + 3] = col[k * 128:(k + 1) * 128]
    return coeff, colT


def _combined_act_table(mybir):
    """Index of an activation-function table holding BOTH Exp and Ln, so a
    single pre-placed load suppresses the per-iteration table reloads that
    otherwise dominate the Activation engine (63 reloads, ~81us/core)."""
    try:
        from concourse.hw_specs import get_activation_tables
        EXP = mybir.ActivationFunctionType.Exp
        LN = mybir.ActivationFunctionType.Ln
        for i, funcs in enumerate(get_activation_tables("gen3").values()):
            if EXP in funcs and LN in funcs:
                return i
    except Exception:
        pass
    return None


def _build_program():
    import concourse.bacc as bacc
    import concourse.mybir as mybir
    from concourse.tile import TileContext
    f32 = mybir.dt.float32
    EXP = mybir.ActivationFunctionType.Exp
    LN = mybir.ActivationFunctionType.Ln

    nc = bacc.Bacc("TRN2")
    b16 = mybir.dt.bfloat16
    static_d = nc.dram_tensor("static", (128, STATIC_F), f32, kind="ExternalInput")
    staticb_d = nc.dram_tensor("staticb", (128, 257), b16, kind="ExternalInput")
    coeff_d = nc.dram_tensor("coeff", (6, N_GAUSS), f32, kind="ExternalInput")
    colt_d = nc.dram_tensor("colt", (128, 3 * NB), b16, kind="ExternalInput")
    out_d = nc.dram_tensor("out", (3, P_CORE), f32, kind="ExternalOutput")

    act_set = _combined_act_table(mybir)
    PT = P_CORE // NP_
    with TileContext(nc) as tc, \
         nc.allow_low_precision("scan/compositing matmuls in bf16; "
                                "rel-err gate is 2e-2"):
        with tc.tile_pool(name="const", bufs=1) as cpool, \
             tc.tile_pool(name="work", bufs=3) as wpool, \
             tc.tile_pool(name="carry", bufs=4) as crpool, \
             tc.tile_pool(name="outp", bufs=2) as opool, \
             tc.tile_pool(name="ps", bufs=2, space="PSUM") as pspool, \
             tc.tile_pool(name="psr", bufs=2, space="PSUM") as psr, \
             tc.tile_pool(name="psc", bufs=2, space="PSUM") as psc:
            if act_set is not None:
                nc.scalar.add_instruction(mybir.InstLoadActFuncSet(
                    name=nc.get_next_instruction_name(),
                    act_func_set_id=act_set, ins=[], outs=[]))
            static = cpool.tile([128, STATIC_F], f32)
            nc.sync.dma_start(out=static[:, :], in_=static_d[:, :])
            staticb = cpool.tile([128, 257], b16)
            nc.sync.dma_start(out=staticb[:, :], in_=staticb_d[:, :])
            coeff = cpool.tile([6, N_GAUSS], f32)
            nc.sync.dma_start(out=coeff[:, :], in_=coeff_d[:, :])
            colt = cpool.tile([128, 3 * NB], b16)
            nc.sync.dma_start(out=colt[:, :], in_=colt_d[:, :])

            for pt in range(PT):
                carryb = crpool.tile([1, NP_], b16, tag="carryb")
                nc.vector.memset(carryb[:, :], 0.0)
                rend = psr.tile([3, NP_], f32, tag="rend")
                for k in range(NB):
                    logits = pspool.tile([128, NP_], f32, tag="logits")
                    nc.tensor.matmul(out=logits[:, :],
                                     lhsT=coeff[:, k * 128:(k + 1) * 128],
                                     rhs=static[0:6, O_BAS + pt * NP_:O_BAS + (pt + 1) * NP_],
                                     start=True, stop=True)
                    am = wpool.tile([128, NP_], f32, tag="am")
                    nc.scalar.activation(out=am[:, :], in_=logits[:, :], func=EXP)
                    l1m = wpool.tile([128, NP_], b16, tag="l1m")
                    nc.scalar.activation(out=l1m[:, :], in_=am[:, :], func=LN,
                                         scale=-1.0, bias=1.0)
                    S = pspool.tile([128, NP_], f32, tag="S")
                    nc.tensor.matmul(out=S[:, :], lhsT=staticb[0:1, 129:257],
                                     rhs=carryb[:, :], start=True, stop=False)
                    nc.tensor.matmul(out=S[:, :], lhsT=staticb[0:128, 0:128],
                                     rhs=l1m[:, :], start=False, stop=True)
                    texcl = wpool.tile([128, NP_], f32, tag="texcl")
                    nc.scalar.activation(out=texcl[:, :], in_=S[:, :], func=EXP)
                    w = wpool.tile([128, NP_], b16, tag="w")
                    nc.vector.tensor_mul(out=w[:, :], in0=am[:, :], in1=texcl[:, :])
                    nc.tensor.matmul(out=rend[:, :],
                                     lhsT=colt[:, 3 * k:3 * k + 3],
                                     rhs=w[:, :], start=(k == 0), stop=(k == NB - 1))
                    if k < NB - 1:
                        csum = psc.tile([1, NP_], f32, tag="csum")
                        nc.tensor.matmul(out=csum[:, :],
                                         lhsT=staticb[0:128, 128:129],
                                         rhs=l1m[:, :], start=True, stop=True)
                        carryb2 = crpool.tile([1, NP_], b16, tag="carryb")
                        nc.vector.tensor_add(out=carryb2[:, :], in0=carryb[:, :],
                                             in1=csum[:, :])
                        carryb = carryb2
                ob = opool.tile([3, NP_], f32, tag="ob")
                nc.vector.tensor_copy(out=ob[:, :], in_=rend[:, :])
                nc.sync.dma_start(out=out_d[:, pt * NP_:(pt + 1) * NP_], in_=ob[:, :])
    nc.finalize()
    return nc


def _make_static():
    """Per-core call-invariant constants: pixel-grid basis, scan mask, ones."""
    ys = np.linspace(-1.0, 1.0, IMG, dtype=np.float32)
    xs = np.linspace(-1.0, 1.0, IMG, dtype=np.float32)
    rows_per_core = IMG // N_CORES
    statics = []
    for c in range(N_CORES):
        gy = np.repeat(ys[c * rows_per_core:(c + 1) * rows_per_core], IMG)
        gx = np.tile(xs, rows_per_core)
        basis = np.stack([gx * gx, gx * gy, gy * gy, gx, gy,
                          np.ones_like(gx)]).astype(np.float32)
        st = np.zeros((128, STATIC_F), np.float32)
        st[0:6, O_BAS:O_BAS + P_CORE] = basis
        st[:, O_TRI:O_TRI + 128] = np.triu(np.ones((128, 128), np.float32), 1)
        st[0, O_ON1:O_ON1 + 128] = 1.0
        st[:, O_ONC] = 1.0
        statics.append(st)
    return np.concatenate(statics, axis=0)           # (8*128, STATIC_F)


def _make_staticb():
    """bf16 masks: strict-upper scan matrix, ones column (block sum),
    ones row (carry broadcast) — all exactly representable in bf16."""
    import ml_dtypes
    st = np.zeros((128, 257), ml_dtypes.bfloat16)
    st[:, 0:128] = np.triu(np.ones((128, 128), np.float32), 1)
    st[:, 128] = 1.0
    st[0, 129:257] = 1.0
    return np.tile(st, (N_CORES, 1))                 # (8*128, 257)


def _get_state():
    """Build the Bass program once, wrap it in a cached jitted shard_map
    dispatch (the per-call rebuild inside run_bass_kernel_spmd's axon
    redirect costs ~200ms of retrace/relower), and park the call-invariant
    constants on the devices so they are never re-uploaded."""
    if _state:
        return _state

    import jax
    from jax.sharding import Mesh, PartitionSpec, NamedSharding
    import warnings
    with warnings.catch_warnings():
        warnings.simplefilter("ignore")
        from jax.experimental.shard_map import shard_map
    from concourse import mybir
    from concourse.bass2jax import (_bass_exec_p, install_neuronx_cc_hook,
                                    partition_id_tensor)

    nc = _build_program()
    install_neuronx_cc_hook()

    partition_name = nc.partition_id_tensor.name if nc.partition_id_tensor else None
    in_names, out_names, out_avals = [], [], []
    for alloc in nc.m.functions[0].allocations:
        if not isinstance(alloc, mybir.MemoryLocationSet):
            continue
        name = alloc.memorylocations[0].name
        if alloc.kind == "ExternalInput":
            if name != partition_name:
                in_names.append(name)
        elif alloc.kind == "ExternalOutput":
            out_names.append(name)
            out_avals.append(jax.core.ShapedArray(tuple(alloc.tensor_shape),
                                                  mybir.dt.np(alloc.dtype)))
    n_params = len(in_names)
    n_outs = len(out_avals)
    in_names_all = in_names + out_names + ([partition_name] if partition_name else [])

    def _body(*args):
        operands = list(args)
        if partition_name is not None:
            operands.append(partition_id_tensor())
        return tuple(_bass_exec_p.bind(
            *operands,
            out_avals=tuple(out_avals),
            in_names=tuple(in_names_all),
            out_names=tuple(out_names),
            lowering_input_output_aliases=(),
            sim_require_finite=True,
            sim_require_nnan=True,
            nc=nc,
        ))

    devices = jax.devices()[:N_CORES]
    mesh = Mesh(np.asarray(devices), ("core",))
    in_specs = (PartitionSpec("core"),) * (n_params + n_outs)
    out_specs = (PartitionSpec("core"),) * n_outs
    # No donation: the kernel DMA-writes every element of `out`, so the
    # pre-zeroed output operands never need to be re-uploaded — park them
    # on the devices once alongside the static constants.
    sharded = jax.jit(
        shard_map(_body, mesh=mesh, in_specs=in_specs, out_specs=out_specs,
                  check_rep=False),
        keep_unused=True)

    sharding = NamedSharding(mesh, PartitionSpec("core"))
    static_dev = jax.device_put(_make_static(), sharding)
    staticb_dev = jax.device_put(_make_staticb(), sharding)
    zeros_dev = [
        jax.device_put(np.zeros((N_CORES * av.shape[0], *av.shape[1:]), av.dtype),
                       sharding)
        for av in out_avals
    ]
    jax.block_until_ready([static_dev, staticb_dev] + zeros_dev)

    _state.update(
        sharded=sharded, in_names=in_names, out_avals=out_avals,
        static_dev=static_dev, staticb_dev=staticb_dev, zeros_dev=zeros_dev,
    )
    return _state


def _run(inputs):
    coeff, colT = _host_prep(inputs["camera_poses"], inputs["positions"],
                             inputs["scales"], inputs["rotations"],
                             inputs["opacity"], inputs["features"])
    import ml_dtypes
    st = _get_state()
    per_call = {
        "coeff": np.tile(coeff, (N_CORES, 1)),                            # (8*6, 1024)
        "colt": np.tile(colT.astype(ml_dtypes.bfloat16), (N_CORES, 1)),   # (8*128, 24)
        "static": st["static_dev"],
        "staticb": st["staticb_dev"],
    }
    args = [per_call[name] for name in st["in_names"]]
    args.extend(st["zeros_dev"])
    out_arrs = st["sharded"](*args)
    res = np.asarray(out_arrs[0])                              # (8*3, P_CORE)
    rows_per_core = IMG // N_CORES
    out = res.reshape(N_CORES, 3, rows_per_core, IMG)
    return np.ascontiguousarray(out.transpose(1, 0, 2, 3)).reshape(1, 3, IMG, IMG)


def kernel(camera_poses, positions, scales, rotations, opacity, features, H, W):
    assert int(H) == IMG and int(W) == IMG
    inputs = {"camera_poses": camera_poses, "positions": positions,
              "scales": scales, "rotations": rotations, "opacity": opacity,
              "features": features}
    # If handed device-resident jax arrays, start all device->host copies in
    # parallel before the blocking np.asarray conversions (sequential fetches
    # each cost a full tunnel round trip).
    for v in inputs.values():
        if hasattr(v, "copy_to_host_async"):
            try:
                v.copy_to_host_async()
            except Exception:
                pass
    inputs = {k: np.asarray(v) for k, v in inputs.items()}
    return _run(inputs)


# revision 22
# speedup vs baseline: 2.0515x; 2.0515x over previous
import sys

sys.path.insert(0, "/opt/trn_rl_repo")

import numpy as np

N_GAUSS = 1024
IMG = 128
NB = 8          # gaussian blocks of 128
NP_ = 512       # pixels per matmul tile (one PSUM bank)
P_CORE = 2048   # pixels per core (16 rows x 128 cols)
N_CORES = 8

# static (call-invariant) tensor layout: free-dim offsets in (128, STATIC_F)
O_BAS = 0                       # basis rows 0..5, P_CORE wide
O_TRI = O_BAS + P_CORE          # strict-upper mask, 128 partitions
O_ON1 = O_TRI + 128             # ones row (partition 0, 128 wide)
O_ONC = O_ON1 + 128             # ones column (128 partitions, 1 wide)
STATIC_F = O_ONC + 1

_state = {}


def _host_prep(camera_poses, positions, scales, rotations, opacity, features):
    pose = np.asarray(camera_poses, np.float32)[0]
    positions = np.asarray(positions, np.float32)
    scales = np.asarray(scales, np.float32)
    rotations = np.asarray(rotations, np.float32)
    opacity = np.asarray(opacity, np.float32)
    features = np.asarray(features, np.float32)
    N = positions.shape[0]

    cam = positions @ pose[:3, :3].T + pose[:3, 3]                         # (N,3)
    depths = cam[:, 2]
    zinv = 1.0 / depths
    px = cam[:, 0] * zinv
    py = cam[:, 1] * zinv

    q = rotations / np.linalg.norm(rotations, axis=1, keepdims=True)
    w, x, y, z = q[:, 0], q[:, 1], q[:, 2], q[:, 3]
    R = np.empty((N, 3, 3), np.float32)
    R[:, 0, 0] = 1 - 2 * (y * y + z * z)
    R[:, 0, 1] = 2 * (x * y - w * z)
    R[:, 0, 2] = 2 * (x * z + w * y)
    R[:, 1, 0] = 2 * (x * y + w * z)
    R[:, 1, 1] = 1 - 2 * (x * x + z * z)
    R[:, 1, 2] = 2 * (y * z - w * x)
    R[:, 2, 0] = 2 * (x * z - w * y)
    R[:, 2, 1] = 2 * (y * z + w * x)
    R[:, 2, 2] = 1 - 2 * (x * x + y * y)

    s2 = (scales * scales)[:, None, :]                                     # (N,1,3)
    cov3d = (R * s2) @ R.transpose(0, 2, 1)                                # (N,3,3)

    Jp = np.zeros((N, 2, 3), np.float32)
    Jp[:, 0, 0] = zinv
    Jp[:, 0, 2] = -cam[:, 0] * zinv * zinv
    Jp[:, 1, 1] = zinv
    Jp[:, 1, 2] = -cam[:, 1] * zinv * zinv
    J = Jp @ pose[:3, :3]                                                  # (N,2,3)
    cov2d = J @ cov3d @ J.transpose(0, 2, 1)                               # (N,2,2)

    a, b = cov2d[:, 0, 0], cov2d[:, 0, 1]
    c, d = cov2d[:, 1, 0], cov2d[:, 1, 1]
    det = a * d - b * c
    i00, i01, i10, i11 = d / det, -b / det, -c / det, a / det

    order = np.argsort(-depths, kind='stable')
    i00, i11 = i00[order], i11[order]
    s = (i01 + i10)[order]
    px, py = px[order], py[order]
    alp = np.maximum(opacity[order, 0], 1e-37)
    col = features[order]                                                  # (N,3)

    # logits = -0.5*m + ln(alpha) as quadratic in (gx, gy):
    #   A gx^2 + B gx gy + C gy^2 + D gx + E gy + F
    coeff = np.empty((6, N), np.float32)
    coeff[0] = -0.5 * i00
    coeff[1] = -0.5 * s
    coeff[2] = -0.5 * i11
    coeff[3] = i00 * px + 0.5 * s * py
    coeff[4] = 0.5 * s * px + i11 * py
    coeff[5] = -0.5 * (i00 * px * px + s * px * py + i11 * py * py) + np.log(alp)

    colT = np.zeros((128, 3 * NB), np.float32)
    for k in range(NB):
        colT[:, 3 * k:3 * k + 3] = col[k * 128:(k + 1) * 128]
    return coeff, colT


def _combined_act_table(mybir):
    """Index of an activation-function table holding BOTH Exp and Ln, so a
    single pre-placed load suppresses the per-iteration table reloads that
    otherwise dominate the Activation engine (63 reloads, ~81us/core)."""
    try:
        from concourse.hw_specs import get_activation_tables
        EXP = mybir.ActivationFunctionType.Exp
        LN = mybir.ActivationFunctionType.Ln
        for i, funcs in enumerate(get_activation_tables("gen3").values()):
            if EXP in funcs and LN in funcs:
                return i
    except Exception:
        pass
    return None


def _build_program():
    import concourse.bacc as bacc
    import concourse.mybir as mybir
    from concourse.tile import TileContext
    f32 = mybir.dt.float32
    EXP = mybir.ActivationFunctionType.Exp
    LN = mybir.ActivationFunctionType.Ln

    nc = bacc.Bacc("TRN2")
    b16 = mybir.dt.bfloat16
    static_d = nc.dram_tensor("static", (128, STATIC_F), f32, kind="ExternalInput")
    staticb_d = nc.dram_tensor("staticb", (128, 257), b16, kind="ExternalInput")
    coeff_d = nc.dram_tensor("coeff", (6, N_GAUSS), f32, kind="ExternalInput")
    colt_d = nc.dram_tensor("colt", (128, 3 * NB), b16, kind="ExternalInput")
    out_d = nc.dram_tensor("out", (3, P_CORE), f32, kind="ExternalOutput")

    act_set = _combined_act_table(mybir)
    PT = P_CORE // NP_
    with TileContext(nc) as tc, \
         nc.allow_low_precision("scan/compositing matmuls in bf16; "
                                "rel-err gate is 2e-2"):
        with tc.tile_pool(name="const", bufs=1) as cpool, \
             tc.tile_pool(name="work", bufs=3) as wpool, \
             tc.tile_pool(name="carry", bufs=4) as crpool, \
             tc.tile_pool(name="outp", bufs=2) as opool, \
             tc.tile_pool(name="ps", bufs=2, space="PSUM") as pspool, \
             tc.tile_pool(name="psr", bufs=2, space="PSUM") as psr, \
             tc.tile_pool(name="psc", bufs=2, space="PSUM") as psc:
            if act_set is not None:
                nc.scalar.add_instruction(mybir.InstLoadActFuncSet(
                    name=nc.get_next_instruction_name(),
                    act_func_set_id=act_set, ins=[], outs=[]))
            static = cpool.tile([128, STATIC_F], f32)
            nc.sync.dma_start(out=static[:, :], in_=static_d[:, :])
            staticb = cpool.tile([128, 257], b16)
            nc.sync.dma_start(out=staticb[:, :], in_=staticb_d[:, :])
            coeff = cpool.tile([6, N_GAUSS], f32)
            nc.sync.dma_start(out=coeff[:, :], in_=coeff_d[:, :])
            colt = cpool.tile([128, 3 * NB], b16)
            nc.sync.dma_start(out=colt[:, :], in_=colt_d[:, :])

            for pt in range(PT):
                carryb = crpool.tile([1, NP_], b16, tag="carryb")
                nc.vector.memset(carryb[:, :], 0.0)
                rend = psr.tile([3, NP_], f32, tag="rend")
                for k in range(NB):
                    logits = pspool.tile([128, NP_], f32, tag="logits")
                    nc.tensor.matmul(out=logits[:, :],
                                     lhsT=coeff[:, k * 128:(k + 1) * 128],
                                     rhs=static[0:6, O_BAS + pt * NP_:O_BAS + (pt + 1) * NP_],
                                     start=True, stop=True)
                    am = wpool.tile([128, NP_], f32, tag="am")
                    nc.scalar.activation(out=am[:, :], in_=logits[:, :], func=EXP)
                    l1m = wpool.tile([128, NP_], b16, tag="l1m")
                    nc.scalar.activation(out=l1m[:, :], in_=am[:, :], func=LN,
                                         scale=-1.0, bias=1.0)
                    S = pspool.tile([128, NP_], f32, tag="S")
                    nc.tensor.matmul(out=S[:, :], lhsT=staticb[0:1, 129:257],
                                     rhs=carryb[:, :], start=True, stop=False)
                    nc.tensor.matmul(out=S[:, :], lhsT=staticb[0:128, 0:128],
                                     rhs=l1m[:, :], start=False, stop=True)
                    texcl = wpool.tile([128, NP_], f32, tag="texcl")
                    nc.scalar.activation(out=texcl[:, :], in_=S[:, :], func=EXP)
                    w = wpool.tile([128, NP_], b16, tag="w")
                    nc.vector.tensor_mul(out=w[:, :], in0=am[:, :], in1=texcl[:, :])
                    nc.tensor.matmul(out=rend[:, :],
                                     lhsT=colt[:, 3 * k:3 * k + 3],
                                     rhs=w[:, :], start=(k == 0), stop=(k == NB - 1))
                    if k < NB - 1:
                        csum = psc.tile([1, NP_], f32, tag="csum")
                        nc.tensor.matmul(out=csum[:, :],
                                         lhsT=staticb[0:128, 128:129],
                                         rhs=l1m[:, :], start=True, stop=True)
                        carryb2 = crpool.tile([1, NP_], b16, tag="carryb")
                        nc.vector.tensor_add(out=carryb2[:, :], in0=carryb[:, :],
                                             in1=csum[:, :])
                        carryb = carryb2
                ob = opool.tile([3, NP_], f32, tag="ob")
                nc.vector.tensor_copy(out=ob[:, :], in_=rend[:, :])
                nc.sync.dma_start(out=out_d[:, pt * NP_:(pt + 1) * NP_], in_=ob[:, :])
    nc.finalize()
    return nc


def _make_static():
    """Per-core call-invariant constants: pixel-grid basis, scan mask, ones."""
    ys = np.linspace(-1.0, 1.0, IMG, dtype=np.float32)
    xs = np.linspace(-1.0, 1.0, IMG, dtype=np.float32)
    rows_per_core = IMG // N_CORES
    statics = []
    for c in range(N_CORES):
        gy = np.repeat(ys[c * rows_per_core:(c + 1) * rows_per_core], IMG)
        gx = np.tile(xs, rows_per_core)
        basis = np.stack([gx * gx, gx * gy, gy * gy, gx, gy,
                          np.ones_like(gx)]).astype(np.float32)
        st = np.zeros((128, STATIC_F), np.float32)
        st[0:6, O_BAS:O_BAS + P_CORE] = basis
        st[:, O_TRI:O_TRI + 128] = np.triu(np.ones((128, 128), np.float32), 1)
        st[0, O_ON1:O_ON1 + 128] = 1.0
        st[:, O_ONC] = 1.0
        statics.append(st)
    return np.concatenate(statics, axis=0)           # (8*128, STATIC_F)


def _make_staticb():
    """bf16 masks: strict-upper scan matrix, ones column (block sum),
    ones row (carry broadcast) — all exactly representable in bf16."""
    import ml_dtypes
    st = np.zeros((128, 257), ml_dtypes.bfloat16)
    st[:, 0:128] = np.triu(np.ones((128, 128), np.float32), 1)
    st[:, 128] = 1.0
    st[0, 129:257] = 1.0
    return np.tile(st, (N_CORES, 1))                 # (8*128, 257)


def _get_state():
    """Build the Bass program once, wrap it in a cached jitted shard_map
    dispatch (the per-call rebuild inside run_bass_kernel_spmd's axon
    redirect costs ~200ms of retrace/relower), and park the call-invariant
    constants on the devices so they are never re-uploaded."""
    if _state:
        return _state

    import jax
    from jax.sharding import Mesh, PartitionSpec, NamedSharding
    import warnings
    with warnings.catch_warnings():
        warnings.simplefilter("ignore")
        from jax.experimental.shard_map import shard_map
    from concourse import mybir
    from concourse.bass2jax import (_bass_exec_p, install_neuronx_cc_hook,
                                    partition_id_tensor)

    nc = _build_program()
    install_neuronx_cc_hook()

    partition_name = nc.partition_id_tensor.name if nc.partition_id_tensor else None
    in_names, out_names, out_avals = [], [], []
    for alloc in nc.m.functions[0].allocations:
        if not isinstance(alloc, mybir.MemoryLocationSet):
            continue
        name = alloc.memorylocations[0].name
        if alloc.kind == "ExternalInput":
            if name != partition_name:
                in_names.append(name)
        elif alloc.kind == "ExternalOutput":
            out_names.append(name)
            out_avals.append(jax.core.ShapedArray(tuple(alloc.tensor_shape),
                                                  mybir.dt.np(alloc.dtype)))
    n_params = len(in_names)
    n_outs = len(out_avals)
    in_names_all = in_names + out_names + ([partition_name] if partition_name else [])

    def _body(*args):
        operands = list(args)
        if partition_name is not None:
            operands.append(partition_id_tensor())
        return tuple(_bass_exec_p.bind(
            *operands,
            out_avals=tuple(out_avals),
            in_names=tuple(in_names_all),
            out_names=tuple(out_names),
            lowering_input_output_aliases=(),
            sim_require_finite=True,
            sim_require_nnan=True,
            nc=nc,
        ))

    devices = jax.devices()[:N_CORES]
    mesh = Mesh(np.asarray(devices), ("core",))
    in_specs = (PartitionSpec("core"),) * (n_params + n_outs)
    out_specs = (PartitionSpec("core"),) * n_outs
    # No donation: the kernel DMA-writes every element of `out`, so the
    # pre-zeroed output operands never need to be re-uploaded — park them
    # on the devices once alongside the static constants.
    sharded = jax.jit(
        shard_map(_body, mesh=mesh, in_specs=in_specs, out_specs=out_specs,
                  check_rep=False),
        keep_unused=True)

    sharding = NamedSharding(mesh, PartitionSpec("core"))
    static_dev = jax.device_put(_make_static(), sharding)
    staticb_dev = jax.device_put(_make_staticb(), sharding)
    zeros_dev = [
        jax.device_put(np.zeros((N_CORES * av.shape[0], *av.shape[1:]), av.dtype),
                       sharding)
        for av in out_avals
    ]
    jax.block_until_ready([static_dev, staticb_dev] + zeros_dev)

    _state.update(
        sharded=sharded, in_names=in_names, out_avals=out_avals,
        static_dev=static_dev, staticb_dev=staticb_dev, zeros_dev=zeros_dev,
    )
    return _state


def _run(inputs):
    coeff, colT = _host_prep(inputs["camera_poses"], inputs["positions"],
                             inputs["scales"], inputs["rotations"],
                             inputs["opacity"], inputs["features"])
    import ml_dtypes
    st = _get_state()
    per_call = {
        "coeff": np.tile(coeff, (N_CORES, 1)),                            # (8*6, 1024)
        "colt": np.tile(colT.astype(ml_dtypes.bfloat16), (N_CORES, 1)),   # (8*128, 24)
        "static": st["static_dev"],
        "staticb": st["staticb_dev"],
    }
    args = [per_call[name] for name in st["in_names"]]
    args.extend(st["zeros_dev"])
    out_arrs = st["sharded"](*args)
    res = np.asarray(out_arrs[0])                              # (8*3, P_CORE)
    rows_per_core = IMG // N_CORES
    out = res.reshape(N_CORES, 3, rows_per_core, IMG)
    return np.ascontiguousarray(out.transpose(1, 0, 2, 3)).reshape(1, 3, IMG, IMG)


def kernel(camera_poses, positions, scales, rotations, opacity, features, H, W):
    assert int(H) == IMG and int(W) == IMG
    inputs = {"camera_poses": camera_poses, "positions": positions,
              "scales": scales, "rotations": rotations, "opacity": opacity,
              "features": features}
    # If handed device-resident jax arrays, start all device->host copies in
    # parallel before the blocking np.asarray conversions (sequential fetches
    # each cost a full tunnel round trip).
    for v in inputs.values():
        if hasattr(v, "copy_to_host_async"):
            try:
                v.copy_to_host_async()
            except Exception:
                pass
    inputs = {k: np.asarray(v) for k, v in inputs.items()}
    return _run(inputs)
